# revision 8
# baseline (speedup 1.0000x reference)
"""Trainium2 Bass kernel for nn_MetricsLoss (cosine-distance metrics loss).

Strategy
--------
Data-parallel over 8 NeuronCores: core c owns rows [c*16384, (c+1)*16384)
= 64 whole contiguous groups of 256 rows.

Host-side preprocessing: within each group, rows are permuted by the stable
argsort of var_lens, so the device computes the cosine distances d already in
sorted-by-var_len order (the only order-dependent terms are the neighbor
monotonicity and pairwise rank losses; every other term is permutation
invariant).  The host also precomputes tiny auxiliary tensors from var_lens /
labels (v_z z-scores, label mask, rank-mask thresholds t = first sorted index
with a strictly larger v).

Device per core (all f32):
  1. Stream z_r, z_v in [128, 2048] tiles.  One fused DVE tensor_tensor_reduce
     per [128,512] slice produces accum = (1+K) - sum(z_r*z_v) = K + d for 128
     rows at once, written as one column of a [128,128] staging tile KDCOL
     (partition = row-within-tile, free = tile index).
  2. Per chunk of 16 tiles: PE transpose -> [16,128] (partition = tile,
     free = row-within-tile) and subtract K -> GW chunk (= d, group-major:
     group g occupies rows 2g, 2g+1).
  3. Rank loss per (group, half): PE broadcast matmul (stationary -1s) puts
     -d_row into PSUM; ACT computes relu(-d_j + (K + d_i)) with the
     per-partition bias taken directly from KDCOL; one fused DVE
     scalar_tensor_tensor computes (iota >= t_i) * R with a free-dim
     accumulate into RANKCOL.
  4. Cheap fused per-tile stats: sum(d-1), sum((d-1)^2), sum(v_z*(d-1)),
     sum((d-1)*mp), neighbor-violation partial sums.
Outputs per core: d (sorted order), stats [128,5], rankcol [128,128].
Host does the final scalar assembly in float64.
"""

import numpy as np

# Problem constants (hardcoded per harness contract).
N, D = 131072, 512
G, NG = 512, 256
MARGIN = 2.0
LAMBDA_CD = 0.0
LAMBDA_CDD = 1.0
LAMBDA_PCC = 1.0
K_MARGIN = 0.02
EPS = 1e-6

N_CORES = 8
ROWS_PC = N // N_CORES            # 16384 rows per core
GROUPS_PC = G // N_CORES          # 64 groups per core
TILES_PC = ROWS_PC // 128         # 128 row-tiles per core
BIG = 4                           # row-tiles per DMA ([128, BIG*512] = 1 MiB)
CHUNK = 16                        # row-tiles per transpose/rank chunk
N_CHUNKS = TILES_PC // CHUNK      # 8
BIGS_PER_CHUNK = CHUNK // BIG     # 4

_CACHE = {}


def _build_bass():
    import concourse.bacc as bacc
    import concourse.mybir as mybir
    import concourse.tile as tile
    from concourse import masks

    f32 = mybir.dt.float32
    AF = mybir.ActivationFunctionType
    OP = mybir.AluOpType

    nc = bacc.Bacc("TRN2", target_bir_lowering=False, debug=False)

    zr = nc.dram_tensor("zr", [ROWS_PC, D], f32, kind="ExternalInput")
    zv = nc.dram_tensor("zv", [ROWS_PC, D], f32, kind="ExternalInput")
    vzw = nc.dram_tensor("vzw", [128, 128], f32, kind="ExternalInput")
    mpw = nc.dram_tensor("mpw", [128, 128], f32, kind="ExternalInput")
    tcol = nc.dram_tensor("tcol", [128, 128], f32, kind="ExternalInput")

    d_out = nc.dram_tensor("d_out", [128, 128], f32, kind="ExternalOutput")
    stats = nc.dram_tensor("stats", [128, 5], f32, kind="ExternalOutput")
    rankcol = nc.dram_tensor("rankcol", [128, 128], f32, kind="ExternalOutput")

    # [ROWS_PC, D] viewed as [NBIG, 128, BIG*D]: big-tile b holds rows
    # 512b..512b+511; free dim is (c k) with c = sub-tile (128 rows), k = D.
    zr_b = zr.ap().rearrange("(b c p) k -> b p c k", c=BIG, p=128)
    zv_b = zv.ap().rearrange("(b c p) k -> b p c k", c=BIG, p=128)

    with tile.TileContext(nc) as tc:
        with (
            tc.tile_pool(name="const", bufs=1) as constp,
            tc.tile_pool(name="zpool", bufs=3) as zp,
            tc.tile_pool(name="persist", bufs=1) as pp,
            tc.tile_pool(name="gwpool", bufs=N_CHUNKS) as gwp,
            tc.tile_pool(name="aux", bufs=2) as auxp,
            tc.tile_pool(name="work", bufs=2) as wp,
            tc.tile_pool(name="pst", bufs=2, space="PSUM") as pstp,
            tc.tile_pool(name="psb", bufs=2, space="PSUM") as psbp,
        ):
            ident = constp.tile([128, 128], f32, tag="ident")
            masks.make_identity(nc, ident[:])
            # per-partition constant columns for activation biases:
            # col 0 = -K_MARGIN, col 1 = -1.0, col 2 = 0.0
            konst = constp.tile([128, 3], f32, tag="konst")
            nc.vector.memset(konst[:, 0:1], 1.0)
            nc.vector.memset(konst[:, 1:2], -1.0)
            nc.vector.memset(konst[:, 2:3], 0.0)
            iota = constp.tile([128, 256], f32, tag="iota")
            nc.gpsimd.iota(
                iota[:], pattern=[[1, 256]], base=0, channel_multiplier=0,
                allow_small_or_imprecise_dtypes=True,
            )

            kdcol = pp.tile([128, TILES_PC], f32, tag="kdcol")   # K + d
            kd2col = pp.tile([128, TILES_PC], f32, tag="kd2col")  # 2K + d
            rkc = pp.tile([128, TILES_PC], f32, tag="rkc")       # rank partials
            tcol_t = pp.tile([128, 128], f32, tag="tcol")
            nc.sync.dma_start(tcol_t[:], tcol.ap())

            for ch in range(N_CHUNKS):
                c0 = ch * CHUNK
                # --- dot products for this chunk ---
                for bb in range(BIGS_PER_CHUNK):
                    b = ch * BIGS_PER_CHUNK + bb
                    zr_tile = zp.tile([128, BIG * D], f32, tag="zr")
                    zv_tile = zp.tile([128, BIG * D], f32, tag="zv")
                    nc.sync.dma_start(
                        zr_tile[:].rearrange("p (c k) -> p c k", c=BIG), zr_b[b])
                    nc.sync.dma_start(
                        zv_tile[:].rearrange("p (c k) -> p c k", c=BIG), zv_b[b])
                    for c in range(BIG):
                        t_idx = b * BIG + c
                        prod = wp.tile([128, D], f32, tag="dotjunk")
                        nc.vector.tensor_tensor(
                            out=prod[:],
                            in0=zr_tile[:, c * D:(c + 1) * D],
                            in1=zv_tile[:, c * D:(c + 1) * D],
                            op=OP.mult,
                        )
                        negp = wp.tile([128, D], f32, tag="dotjunk2")
                        nc.vector.tensor_scalar(
                            out=negp[:], in0=prod[:], scalar1=-1.0, scalar2=0.0,
                            op0=OP.mult, op1=OP.add,
                            accum_out=kdcol[:, t_idx:t_idx + 1],
                        )

                nc.vector.tensor_scalar(
                    out=kd2col[:, c0:c0 + CHUNK], in0=kdcol[:, c0:c0 + CHUNK],
                    scalar1=K_MARGIN, scalar2=0.0, op0=OP.add, op1=OP.add,
                )
                # --- transpose chunk: [128, 16] -> [16, 128]; gw = d ---
                pst = pstp.tile([CHUNK, 128], f32, tag="pst")
                nc.tensor.transpose(pst[:], kdcol[:, c0:c0 + CHUNK], ident[:])
                gw_c = gwp.tile([CHUNK, 128], f32, tag=f"gw{ch}")
                nc.scalar.activation(
                    gw_c[:], pst[:], AF.Identity,
                    bias=konst[0:CHUNK, 0:1], scale=1.0,
                )
                nc.sync.dma_start(d_out.ap()[c0:c0 + CHUNK, :], gw_c[:])

                # --- per-chunk stats ---
                vz_c = auxp.tile([CHUNK, 128], f32, tag="vz")
                mp_c = auxp.tile([CHUNK, 128], f32, tag="mp")
                nc.sync.dma_start(vz_c[:], vzw.ap()[c0:c0 + CHUNK, :])
                nc.sync.dma_start(mp_c[:], mpw.ap()[c0:c0 + CHUNK, :])
                st_c = auxp.tile([CHUNK, 5], f32, tag="st")
                sj = wp.tile([CHUNK, 128], f32, tag="sj")
                nc.vector.tensor_scalar(
                    out=sj[:], in0=gw_c[:], scalar1=1.0, scalar2=0.0,
                    op0=OP.subtract, op1=OP.add, accum_out=st_c[:, 0:1],
                )
                sj2 = wp.tile([CHUNK, 128], f32, tag="sj2")
                nc.scalar.activation(
                    sj2[:], gw_c[:], AF.Square, bias=konst[0:CHUNK, 1:2],
                    scale=1.0, accum_out=st_c[:, 1:2],
                )
                dm1 = wp.tile([CHUNK, 128], f32, tag="dm1")
                nc.vector.tensor_scalar(
                    out=dm1[:], in0=gw_c[:], scalar1=1.0, scalar2=0.0,
                    op0=OP.subtract, op1=OP.add,
                )
                sj3 = wp.tile([CHUNK, 128], f32, tag="sj3")
                nc.vector.tensor_tensor(
                    out=sj3[:], in0=dm1[:], in1=vz_c[:], op=OP.mult)
                sj3b = wp.tile([CHUNK, 128], f32, tag="sj3b")
                nc.vector.tensor_scalar(
                    out=sj3b[:], in0=sj3[:], scalar1=1.0, scalar2=0.0,
                    op0=OP.mult, op1=OP.add, accum_out=st_c[:, 2:3],
                )
                sj4 = wp.tile([CHUNK, 128], f32, tag="sj4")
                nc.vector.tensor_tensor(
                    out=sj4[:], in0=dm1[:], in1=mp_c[:], op=OP.mult)
                sj4b = wp.tile([CHUNK, 128], f32, tag="sj4b")
                nc.vector.tensor_scalar(
                    out=sj4b[:], in0=sj4[:], scalar1=1.0, scalar2=0.0,
                    op0=OP.mult, op1=OP.add, accum_out=st_c[:, 3:4],
                )
                ndk = wp.tile([CHUNK, 127], f32, tag="ndk")
                nc.vector.tensor_scalar(
                    out=ndk[:], in0=gw_c[:, 0:127], scalar1=K_MARGIN,
                    scalar2=0.0, op0=OP.add, op1=OP.add,
                )
                nd = wp.tile([CHUNK, 127], f32, tag="nd")
                nc.vector.tensor_tensor(
                    out=nd[:], in0=ndk[:], in1=gw_c[:, 1:128], op=OP.subtract)
                sj5 = wp.tile([CHUNK, 127], f32, tag="sj5")
                nc.scalar.activation(
                    sj5[:], nd[:], AF.Relu, bias=konst[0:CHUNK, 2:3],
                    accum_out=st_c[:, 4:5],
                )
                nc.sync.dma_start(stats.ap()[c0:c0 + CHUNK, :], st_c[:])

                # --- rank loss for the 8 groups of this chunk ---
                for gloc in range(CHUNK // 2):
                    row0 = 2 * gloc
                    col0 = c0 + row0
                    psb = psbp.tile([128, 256], f32, tag="psb")
                    # transpose+broadcast in one matmul: stationary is a
                    # stride-0 broadcast of one kdcol column; out[m, n] =
                    # kdcol[n, col] = K + d_j for every partition m.
                    nc.tensor.matmul(
                        psb[:, 0:128],
                        kdcol[:, col0:col0 + 1].to_broadcast((128, 128)),
                        ident[:], start=True, stop=True,
                    )
                    nc.tensor.matmul(
                        psb[:, 128:256],
                        kdcol[:, col0 + 1:col0 + 2].to_broadcast((128, 128)),
                        ident[:], start=True, stop=True,
                    )
                    for h in range(2):
                        col = col0 + h
                        r_t = wp.tile([128, 256], f32, tag="relu")
                        nc.scalar.activation(
                            r_t[:], psb[:], AF.Relu,
                            bias=kd2col[:, col:col + 1], scale=-1.0,
                        )
                        msk = wp.tile([128, 256], f32, tag="msk")
                        nc.vector.tensor_scalar(
                            out=msk[:], in0=iota[:],
                            scalar1=tcol_t[:, col:col + 1], scalar2=0.0,
                            op0=OP.is_ge, op1=OP.add,
                        )
                        mr = wp.tile([128, 256], f32, tag="mr")
                        nc.gpsimd.tensor_tensor(
                            out=mr[:], in0=msk[:], in1=r_t[:], op=OP.mult)
                        rj = wp.tile([128, 256], f32, tag="rj")
                        nc.scalar.activation(
                            rj[:], mr[:], AF.Identity, bias=konst[:, 2:3],
                            scale=1.0, accum_out=rkc[:, col:col + 1],
                        )

            nc.sync.dma_start(rankcol.ap(), rkc[:])

    nc.compile()
    return nc


def _get_nc():
    if "nc" not in _CACHE:
        _CACHE["nc"] = _build_bass()
    return _CACHE["nc"]


def _prep_host(z_r, z_v, labels, var_lens):
    """Host-side preprocessing: permutation + aux tensors."""
    v_g = var_lens.reshape(G, NG).astype(np.float64)
    order = np.argsort(v_g, axis=1, kind="stable")          # [G, NG]
    v_s = np.take_along_axis(v_g, order, axis=1)            # sorted v

    # t[g, i] = #{j : v_s[g, j] <= v_s[g, i]} = first index with larger v
    t = (v_s[:, None, :] <= v_s[:, :, None]).sum(axis=2)    # [G, NG] int
    cnt = (NG - t).sum(axis=1)                              # [G] pairs with dv>0

    # v_z in the reference's formula (computed in f64; fed to device as f32)
    vmean = v_g.mean(axis=1, keepdims=True)
    vstd = v_g.std(axis=1, ddof=1)                          # [G]
    vz = (v_g - vmean) / (vstd + EPS)[:, None]
    vz_s = np.take_along_axis(vz, order, axis=1)            # sorted order

    lab = np.asarray(labels).astype(np.int64)
    mp = (lab == 1).astype(np.float32)                      # [N]
    mp_g = mp.reshape(G, NG)
    mp_s = np.take_along_axis(mp_g, order, axis=1)          # sorted order

    perm_flat = (np.arange(G)[:, None] * NG + order).ravel()  # global row perm

    return {
        "order": order, "v_s": v_s, "t": t, "cnt": cnt,
        "vz_s": vz_s, "vstd": vstd, "mp_s": mp_s, "lab": lab,
        "perm_flat": perm_flat,
        "sum_vz": vz_s.sum(axis=1), "sum_vz2": (vz_s ** 2).sum(axis=1),
        "npos_g": mp_g.sum(axis=1),
    }


def _make_in_maps(z_r, z_v, prep):
    perm = prep["perm_flat"]
    zr_p = np.ascontiguousarray(z_r[perm])                  # [N, D] permuted
    zv_p = np.ascontiguousarray(z_v[perm])
    vz_flat = prep["vz_s"].astype(np.float32).ravel()
    mp_flat = prep["mp_s"].astype(np.float32).ravel()
    t_flat = prep["t"].astype(np.float32).ravel()

    in_maps = []
    for c in range(N_CORES):
        sl = slice(c * ROWS_PC, (c + 1) * ROWS_PC)
        in_maps.append({
            "zr": zr_p[sl],
            "zv": zv_p[sl],
            "vzw": vz_flat[sl].reshape(128, 128),
            "mpw": mp_flat[sl].reshape(128, 128),
            # tcol layout: [lane l, tile tt] = t_flat[128*tt + l]
            "tcol": np.ascontiguousarray(t_flat[sl].reshape(128, 128).T),
        })
    return in_maps


def _assemble(results, prep):
    """Combine per-core device outputs into the final loss values (f64)."""
    d_sorted = np.concatenate(
        [np.asarray(r["d_out"], np.float64).reshape(ROWS_PC) for r in results])
    st = np.concatenate(
        [np.asarray(r["stats"], np.float64) for r in results])       # [1024, 5]
    rkc = np.concatenate(
        [np.asarray(r["rankcol"], np.float64).sum(axis=0) for r in results])

    # per-group sums: tiles 2g, 2g+1 belong to group g
    stg = st.reshape(G, 2, 5).sum(axis=1)                            # [G, 5]
    s1, s2, s3, s4, ns_dev = stg.T
    pair_sum = rkc.reshape(G, 2).sum(axis=1)                         # [G]

    # ---- group losses ----
    mu = 1.0 + s1 / NG
    ss = s2 - s1 * s1 / NG                                 # sum((d - mu)^2)
    ss = np.maximum(ss, 0.0)
    d_std = np.sqrt(ss / (NG - 1))
    s = d_std + EPS
    sum_vz = prep["sum_vz"]
    sum_vz2 = prep["sum_vz2"]
    sum_vz_dz = (s3 + (1.0 - mu) * sum_vz) / s
    sum_dz2 = ss / (s * s)
    corr = (sum_vz2 - 2.0 * sum_vz_dz + sum_dz2) / NG
    corr = np.where((prep["vstd"] > 0) & (d_std > 0), corr, 0.0)

    ds_g = d_sorted.reshape(G, NG)
    seam = np.maximum(ds_g[:, 127] - ds_g[:, 128] + K_MARGIN, 0.0)
    neigh = (ns_dev + seam) / (NG - 1)

    cnt = prep["cnt"]
    rank = np.where(cnt > 0, pair_sum / np.maximum(cnt, 1.0), 0.0)

    l_pcc = (corr + neigh + rank).mean()

    # ---- global masked means ----
    npos_g = prep["npos_g"]
    sum_d = (s1 + NG).sum()                                # total sum of d
    sum_dmp = (s4 + npos_g).sum()                          # sum of d * mp
    npos = npos_g.sum()
    nb = N - npos
    db_mean = (sum_d - sum_dmp) / max(nb, 1.0)
    dp_mean = sum_dmp / max(npos, 1.0)
    l_cd = db_mean if nb > 0 else 0.0
    l_cdd = max(MARGIN + db_mean - dp_mean, 0.0) if (nb > 0 and npos > 0) else 0.0

    total = LAMBDA_CD * l_cd + LAMBDA_CDD * l_cdd + LAMBDA_PCC * l_pcc

    # un-permute d
    d_full = np.empty(N, np.float32)
    d_full[prep["perm_flat"]] = d_sorted.astype(np.float32)

    return (np.float32(total), np.float32(l_cdd), np.float32(l_pcc), d_full)


def kernel(z_r, z_v, labels, groups, var_lens):
    from concourse.bass_utils import run_bass_kernel_spmd

    z_r = np.asarray(z_r, dtype=np.float32)
    z_v = np.asarray(z_v, dtype=np.float32)
    var_lens = np.asarray(var_lens)

    prep = _prep_host(z_r, z_v, labels, var_lens)
    in_maps = _make_in_maps(z_r, z_v, prep)
    nc = _get_nc()
    res = run_bass_kernel_spmd(nc, in_maps, core_ids=list(range(N_CORES)))
    return _assemble(res.results, prep)
